# revision 12
# baseline (speedup 1.0000x reference)
"""Trainium2 Bass kernel: LSTM (B=2048, T=1024, I=4, H=16) + sigmoid dense head.

Sharding: pure data parallel, batch split over 8 cores (256 each = 2 chains x 128).

Batch-on-partitions orientation: the gate matmul is z_slot^T @ W with the
z-ring slice [21, 128] as the *stationary* lhsT and the weight matrix
[21, 65] as the moving rhs, so gates land [128 batch, 65 gate-cols] in PSUM.
Every elementwise op is then a full-128-lane column-sliced op (base partition
0 everywhere: no partition-base legality issues, bf16 2x packing applies) and
ONE sigmoid ACT covers all 4 gates + the dense-head pre-activation
y = 2*W_d h~ + b_d (rhs col 64, zero extra ops).

Per chain-step: MM -> ACT sigma_all -> DVE q=(sg-.5)*si -> DVE pb=sf*c ->
DVE cn=q+pb -> ACT u=sigma(4c~) -> DVE h~=(u-.5)*so -> PE transpose
[128,16]->[16,128] -> DVE copy PSUM->SBUF z-ring.

sigma(y) columns are DMA-gathered from the bf16 sigma-ring straight to DRAM
(batch-major ot [BCORE, T]) once per 8 steps.
State scalings: c~ = c/2, h~ = h/2 (absorbed into weights); tanh via
sigma(2x) identities so only the Sigmoid LUT is ever used.
"""
import sys
sys.path.insert(0, "/opt/trn_rl_repo")
import numpy as np
from contextlib import ExitStack

import concourse.bass as bass
import concourse.tile as tile
from concourse import bacc, mybir

F32 = mybir.dt.float32
BF16 = mybir.dt.bfloat16
AF = mybir.ActivationFunctionType
OP = mybir.AluOpType

B, T, I, H = 2048, 1024, 4, 16
NCORES = 8
BCORE = B // NCORES          # 256
NB = 128                     # batch per chain
NCH = 2                      # chains per core
KD = 21                      # z rows: 16 h~ + 4 x + 1 ones
GCOL = 65                    # rhs cols: f@0 i@16 o@32 g@48 y@64
SW = 66                      # sigma ring slot width (pad for 4B alignment)
KSLOT = 257                  # Z ring slots (2*STAGE+1)
STAGE = 128                  # x staging granularity (steps)
RS = 16                      # sigma ring slots (y gather groups of 8)

_CACHE = {}


def _emit_core(nc, t_steps):
    wg = nc.dram_tensor("wg", [KD, GCOL], BF16, kind="ExternalInput").ap()
    eye = nc.dram_tensor("eye", [128, 128], BF16, kind="ExternalInput").ap()
    xt = nc.dram_tensor("xt", [t_steps, I, BCORE], BF16, kind="ExternalInput").ap()
    ones = nc.dram_tensor("ones", [1, KSLOT * NB], BF16, kind="ExternalInput").ap()
    h_in = nc.dram_tensor("h_in", [16, BCORE], BF16, kind="ExternalInput").ap()
    c_in = nc.dram_tensor("c_in", [BCORE, 16], F32, kind="ExternalInput").ap()
    h_out = nc.dram_tensor("h_out", [16, BCORE], BF16, kind="ExternalOutput").ap()
    c_out = nc.dram_tensor("c_out", [BCORE, 16], F32, kind="ExternalOutput").ap()
    ot = nc.dram_tensor("ot", [BCORE, t_steps], BF16, kind="ExternalOutput").ap()

    with tile.TileContext(nc) as tc, ExitStack() as ctx:
        const = ctx.enter_context(tc.tile_pool(name="const", bufs=1))
        zpool = ctx.enter_context(tc.tile_pool(name="zp", bufs=1))
        spool = ctx.enter_context(tc.tile_pool(name="sp", bufs=1))
        work = ctx.enter_context(tc.tile_pool(name="wk", bufs=4))
        gpool = ctx.enter_context(tc.tile_pool(name="gp", bufs=2, space="PSUM"))
        tpool = ctx.enter_context(tc.tile_pool(name="tp", bufs=2, space="PSUM"))

        twg = const.tile([KD, GCOL], BF16)
        teye = const.tile([128, 128], BF16)
        nc.sync.dma_start(twg[:], wg[:])
        nc.sync.dma_start(teye[:], eye[:])

        # Z rings: rows 0:16 h~ (bf16), rows 16:20 x, row 20 ones
        z = [zpool.tile([KD, KSLOT * NB], BF16, name=f"z{c}") for c in range(NCH)]
        for c in range(NCH):
            nc.sync.dma_start(z[c][0:16, 0:NB], h_in[:, c * NB:(c + 1) * NB])
            nc.sync.dma_start(z[c][20:21, :], ones[:])

        # sigma rings: [128 batch, RS slots x 66 cols]; cols f i o g y pad
        S = [spool.tile([128, RS * SW], BF16, name=f"s{c}") for c in range(NCH)]

        c_cur = []
        for c in range(NCH):
            ci = work.tile([128, 16], F32, tag=f"c{c}", name=f"ci{c}")
            nc.sync.dma_start(ci[:], c_in[c * NB:(c + 1) * NB, :])
            c_cur.append(ci)

        def stage_x(c, t0, nsteps):
            s0 = t0 % KSLOT
            runs = []
            if s0 + nsteps <= KSLOT:
                runs.append((s0, t0, nsteps))
            else:
                n1 = KSLOT - s0
                runs.append((s0, t0, n1))
                runs.append((0, t0 + n1, nsteps - n1))
            for (sl, tt, ln) in runs:
                src = xt[tt:tt + ln, :, c * NB:(c + 1) * NB].rearrange("t i b -> i t b")
                dst = z[c][16:20, sl * NB:(sl + ln) * NB].rearrange(
                    "i (s b) -> i s b", s=ln)
                nc.sync.dma_start(dst, src)

        for c in range(NCH):
            stage_x(c, 0, min(STAGE, t_steps))

        for t in range(t_steps):
            if t % STAGE == 0 and t + STAGE < t_steps:
                for c in range(NCH):
                    stage_x(c, t + STAGE, min(STAGE, t_steps - t - STAGE))
            sl = t % KSLOT
            nsl = (t + 1) % KSLOT
            ss = t % RS
            for c in range(NCH):
                g = gpool.tile([128, GCOL], F32, tag=f"g{c}", name=f"g{c}_{t}")
                nc.tensor.matmul(g[:], z[c][:, sl * NB:(sl + 1) * NB], twg[:],
                                 start=True, stop=True)
                sv = S[c][:, ss * SW:ss * SW + GCOL]
                nc.scalar.activation(sv[:], g[:], AF.Sigmoid)
                sf = S[c][:, ss * SW + 0:ss * SW + 16]
                si = S[c][:, ss * SW + 16:ss * SW + 32]
                so = S[c][:, ss * SW + 32:ss * SW + 48]
                sg = S[c][:, ss * SW + 48:ss * SW + 64]
                q = work.tile([128, 16], BF16, tag=f"q{c}", name=f"q{c}_{t}")
                nc.vector.scalar_tensor_tensor(
                    q[:], sg, 0.5, si, op0=OP.subtract, op1=OP.mult)
                pb = work.tile([128, 16], F32, tag=f"p{c}", name=f"p{c}_{t}")
                nc.vector.scalar_tensor_tensor(
                    pb[:], sf, 0.0, c_cur[c][:], op0=OP.add, op1=OP.mult)
                cn = work.tile([128, 16], F32, tag=f"c{c}", name=f"cn{c}_{t}")
                nc.vector.scalar_tensor_tensor(
                    cn[:], q[:], 0.0, pb[:], op0=OP.add, op1=OP.add)
                u = work.tile([128, 16], BF16, tag=f"u{c}", name=f"u{c}_{t}")
                nc.scalar.activation(u[:], cn[:], AF.Sigmoid, scale=4.0)
                hh = work.tile([128, 16], BF16, tag=f"h{c}", name=f"h{c}_{t}")
                nc.vector.scalar_tensor_tensor(
                    hh[:], u[:], 0.5, so, op0=OP.subtract, op1=OP.mult)
                tp = tpool.tile([16, NB], BF16, tag=f"t{c}", name=f"tp{c}_{t}")
                nc.tensor.transpose(tp[:], hh[:], teye[:])
                nc.vector.tensor_scalar_add(
                    z[c][0:16, nsl * NB:(nsl + 1) * NB], tp[:], 0.0)
                c_cur[c] = cn

            # gather sigma(y) columns (slot s holds y_{t(s)-1}) to DRAM
            if t % 8 == 7:
                s0 = (t - 7) % RS   # always 0 or 8: contiguous run of 8
                for c in range(NCH):
                    if t == 7:  # slot 0 of chunk = y_{-1}: skip it
                        src = S[c][:, 1 * SW + 64:7 * SW + 65:SW]
                        dst = ot[c * NB:(c + 1) * NB, 0:7]
                    else:
                        src = S[c][:, (s0 * SW + 64):((s0 + 7) * SW + 65):SW]
                        dst = ot[c * NB:(c + 1) * NB, t - 8:t]
                    nc.sync.dma_start(dst, src)

        # trailing y_{t_steps-1} = sigma(2 wd h~_last + bd)
        fsl = t_steps % KSLOT
        for c in range(NCH):
            gt = gpool.tile([128, 1], F32, tag=f"g{c}", name=f"gt{c}")
            nc.tensor.matmul(gt[:], z[c][:, fsl * NB:(fsl + 1) * NB],
                             twg[:, 64:65], start=True, stop=True)
            st = work.tile([128, 1], BF16, tag=f"q{c}", name=f"st{c}")
            nc.scalar.activation(st[:], gt[:], AF.Sigmoid)
            nc.sync.dma_start(ot[c * NB:(c + 1) * NB, t_steps - 1:t_steps], st[:])

        for c in range(NCH):
            nc.sync.dma_start(h_out[:, c * NB:(c + 1) * NB],
                              z[c][0:16, fsl * NB:(fsl + 1) * NB])
            nc.sync.dma_start(c_out[c * NB:(c + 1) * NB, :], c_cur[c][:])


def _prep_host(W_ih, W_hh, b_ih, b_hh, W_d, b_d):
    # PyTorch gate order blocks of 16: [i, f, g, o]
    Wi, Wf, Wgt, Wo = W_ih[0:16], W_ih[16:32], W_ih[32:48], W_ih[48:64]
    Ui, Uf, Ugt, Uo = W_hh[0:16], W_hh[16:32], W_hh[32:48], W_hh[48:64]
    bb = b_ih + b_hh
    bi, bf, bgt, bo = bb[0:16], bb[16:32], bb[32:48], bb[48:64]

    wg = np.zeros((KD, GCOL), np.float32)

    def put(base, Wx, Ux, bx, scale):
        wg[0:16, base:base + 16] = (2.0 * scale) * Ux.T   # h~ = h/2
        wg[16:20, base:base + 16] = scale * Wx.T
        wg[20, base:base + 16] = scale * bx

    put(0, Wf, Uf, bf, 1.0)
    put(16, Wi, Ui, bi, 1.0)
    put(32, Wo, Uo, bo, 1.0)
    put(48, Wgt, Ugt, bgt, 2.0)   # sigma(2 glin)
    wg[0:16, 64] = 2.0 * W_d[0]   # y = 2 wd h~ + bd
    wg[20, 64] = float(b_d[0])
    return wg


def _get_compiled(t_steps):
    key = ("nc", t_steps)
    if key not in _CACHE:
        nc = bacc.Bacc("TRN2", target_bir_lowering=False, debug=False)
        _emit_core(nc, t_steps)
        nc.compile()
        _CACHE[key] = nc
    return _CACHE[key]


def kernel(x, W_ih, W_hh, b_ih, b_hh, W_d, b_d, _trace=False, _t_steps=T):
    import ml_dtypes
    from concourse.bass_utils import run_bass_kernel_spmd

    x = np.asarray(x, dtype=np.float32)
    ts = _t_steps
    wg = _prep_host(
        np.asarray(W_ih, np.float32), np.asarray(W_hh, np.float32),
        np.asarray(b_ih, np.float32), np.asarray(b_hh, np.float32),
        np.asarray(W_d, np.float32), np.asarray(b_d, np.float32))
    wg16 = wg.astype(ml_dtypes.bfloat16)
    eye16 = np.eye(128, dtype=ml_dtypes.bfloat16)

    # x [B, ts, I] -> [ts, I, B] bf16
    xtr16 = np.ascontiguousarray(
        x[:, 0:ts, :].transpose(1, 2, 0)).astype(ml_dtypes.bfloat16)

    CH = 512 if ts % 512 == 0 else ts
    nchunk = ts // CH
    nc = _get_compiled(CH)
    _ONES = np.ones((1, KSLOT * NB), ml_dtypes.bfloat16)
    h_st = [np.zeros((16, BCORE), ml_dtypes.bfloat16) for _ in range(NCORES)]
    c_st = [np.zeros((BCORE, 16), np.float32) for _ in range(NCORES)]
    out = np.empty((B, ts, 1), np.float32)
    total_ns = 0
    for ck in range(nchunk):
        in_maps = []
        for cix in range(NCORES):
            in_maps.append({
                "wg": wg16, "eye": eye16, "ones": _ONES,
                "h_in": h_st[cix], "c_in": c_st[cix],
                "xt": np.ascontiguousarray(
                    xtr16[ck * CH:(ck + 1) * CH, :,
                          cix * BCORE:(cix + 1) * BCORE]),
            })
        res = run_bass_kernel_spmd(nc, in_maps, core_ids=list(range(NCORES)),
                                   trace=_trace)
        for cix in range(NCORES):
            out[cix * BCORE:(cix + 1) * BCORE,
                ck * CH:(ck + 1) * CH, 0] = res.results[cix]["ot"]
            h_st[cix] = res.results[cix]["h_out"]
            c_st[cix] = res.results[cix]["c_out"]
        if res.exec_time_ns:
            total_ns += res.exec_time_ns
    kernel._last_exec_ns = total_ns or None
    return out


# revision 13
# speedup vs baseline: 1.1339x; 1.1339x over previous
"""Trainium2 Bass kernel: LSTM (B=2048, T=1024, I=4, H=16) + sigmoid dense head.

Sharding: pure data parallel, batch split over 8 cores (256 each = 2 chains x 128).

Batch-on-partitions orientation: the gate matmul is z_slot^T @ W with the
z-ring slice [21, 128] as the *stationary* lhsT and the weight matrix
[21, 65] as the moving rhs, so gates land [128 batch, 65 gate-cols] in PSUM.
Every elementwise op is then a full-128-lane column-sliced op (base partition
0 everywhere: no partition-base legality issues, bf16 2x packing applies) and
ONE sigmoid ACT covers all 4 gates + the dense-head pre-activation
y = 2*W_d h~ + b_d (rhs col 64, zero extra ops).

Per chain-step: MM -> ACT sigma_all -> DVE q=(sg-.5)*si -> DVE pb=sf*c ->
DVE cn=q+pb -> ACT u=sigma(4c~) -> DVE h~=(u-.5)*so -> PE transpose
[128,16]->[16,128] -> DVE copy PSUM->SBUF z-ring.

sigma(y) columns are DMA-gathered from the bf16 sigma-ring straight to DRAM
(batch-major ot [BCORE, T]) once per 8 steps.
State scalings: c~ = c/2, h~ = h/2 (absorbed into weights); tanh via
sigma(2x) identities so only the Sigmoid LUT is ever used.
"""
import sys
sys.path.insert(0, "/opt/trn_rl_repo")
import numpy as np
from contextlib import ExitStack

import concourse.bass as bass
import concourse.tile as tile
from concourse import bacc, mybir

F32 = mybir.dt.float32
BF16 = mybir.dt.bfloat16
AF = mybir.ActivationFunctionType
OP = mybir.AluOpType

B, T, I, H = 2048, 1024, 4, 16
NCORES = 8
BCORE = B // NCORES          # 256
NB = 128                     # batch per chain
NCH = 2                      # chains per core
KD = 21                      # z rows: 16 h~ + 4 x + 1 ones
GCOL = 65                    # rhs cols: f@0 i@16 o@32 g@48 y@64
SW = 66                      # sigma ring slot width (pad for 4B alignment)
KSLOT = 257                  # Z ring slots (2*STAGE+1)
STAGE = 128                  # x staging granularity (steps)
RS = 16                      # sigma ring slots (y gather groups of 8)

_CACHE = {}


def _emit_core(nc, t_steps):
    wg = nc.dram_tensor("wg", [KD, GCOL], BF16, kind="ExternalInput").ap()
    eye = nc.dram_tensor("eye", [128, 128], BF16, kind="ExternalInput").ap()
    xt = nc.dram_tensor("xt", [t_steps, I, BCORE], BF16, kind="ExternalInput").ap()
    ones = nc.dram_tensor("ones", [1, KSLOT * NB], BF16, kind="ExternalInput").ap()
    h_in = nc.dram_tensor("h_in", [16, BCORE], BF16, kind="ExternalInput").ap()
    c_in = nc.dram_tensor("c_in", [BCORE, 16], F32, kind="ExternalInput").ap()
    h_out = nc.dram_tensor("h_out", [16, BCORE], BF16, kind="ExternalOutput").ap()
    c_out = nc.dram_tensor("c_out", [BCORE, 16], F32, kind="ExternalOutput").ap()
    ot = nc.dram_tensor("ot", [BCORE, t_steps], BF16, kind="ExternalOutput").ap()

    with tile.TileContext(nc) as tc, ExitStack() as ctx:
        const = ctx.enter_context(tc.tile_pool(name="const", bufs=1))
        zpool = ctx.enter_context(tc.tile_pool(name="zp", bufs=1))
        spool = ctx.enter_context(tc.tile_pool(name="sp", bufs=1))
        work = ctx.enter_context(tc.tile_pool(name="wk", bufs=4))
        gpool = ctx.enter_context(tc.tile_pool(name="gp", bufs=2, space="PSUM"))
        tpool = ctx.enter_context(tc.tile_pool(name="tp", bufs=2, space="PSUM"))

        twg = const.tile([KD, GCOL], BF16)
        teye = const.tile([128, 128], BF16)
        nc.sync.dma_start(twg[:], wg[:])
        nc.sync.dma_start(teye[:], eye[:])

        # Z rings: rows 0:16 h~ (bf16), rows 16:20 x, row 20 ones
        z = [zpool.tile([KD, KSLOT * NB], BF16, name=f"z{c}") for c in range(NCH)]
        for c in range(NCH):
            nc.sync.dma_start(z[c][0:16, 0:NB], h_in[:, c * NB:(c + 1) * NB])
            nc.sync.dma_start(z[c][20:21, :], ones[:])

        # sigma rings: [128 batch, RS slots x 66 cols]; cols f i o g y pad
        S = [spool.tile([128, RS * SW], BF16, name=f"s{c}") for c in range(NCH)]

        c_cur = []
        for c in range(NCH):
            ci = work.tile([128, 16], F32, tag=f"c{c}", name=f"ci{c}")
            nc.sync.dma_start(ci[:], c_in[c * NB:(c + 1) * NB, :])
            c_cur.append(ci)

        def stage_x(c, t0, nsteps):
            s0 = t0 % KSLOT
            runs = []
            if s0 + nsteps <= KSLOT:
                runs.append((s0, t0, nsteps))
            else:
                n1 = KSLOT - s0
                runs.append((s0, t0, n1))
                runs.append((0, t0 + n1, nsteps - n1))
            for (sl, tt, ln) in runs:
                src = xt[tt:tt + ln, :, c * NB:(c + 1) * NB].rearrange("t i b -> i t b")
                dst = z[c][16:20, sl * NB:(sl + ln) * NB].rearrange(
                    "i (s b) -> i s b", s=ln)
                nc.sync.dma_start(dst, src)

        for c in range(NCH):
            stage_x(c, 0, min(STAGE, t_steps))

        for t in range(t_steps):
            if t % STAGE == 0 and t + STAGE < t_steps:
                for c in range(NCH):
                    stage_x(c, t + STAGE, min(STAGE, t_steps - t - STAGE))
            sl = t % KSLOT
            nsl = (t + 1) % KSLOT
            ss = t % RS
            for c in range(NCH):
                g = gpool.tile([128, GCOL], F32, tag=f"g{c}", name=f"g{c}_{t}")
                nc.tensor.matmul(g[:], z[c][:, sl * NB:(sl + 1) * NB], twg[:],
                                 start=True, stop=True)
                sv = S[c][:, ss * SW:ss * SW + GCOL]
                nc.scalar.activation(sv[:], g[:], AF.Sigmoid)
                sf = S[c][:, ss * SW + 0:ss * SW + 16]
                si = S[c][:, ss * SW + 16:ss * SW + 32]
                so = S[c][:, ss * SW + 32:ss * SW + 48]
                sg = S[c][:, ss * SW + 48:ss * SW + 64]
                q = work.tile([128, 16], BF16, tag=f"q{c}", name=f"q{c}_{t}")
                nc.vector.scalar_tensor_tensor(
                    q[:], sg, 0.5, si, op0=OP.subtract, op1=OP.mult)
                pb = work.tile([128, 16], F32, tag=f"p{c}", name=f"p{c}_{t}")
                nc.vector.scalar_tensor_tensor(
                    pb[:], sf, 0.0, c_cur[c][:], op0=OP.add, op1=OP.mult)
                cn = work.tile([128, 16], F32, tag=f"c{c}", name=f"cn{c}_{t}")
                nc.gpsimd.tensor_tensor(cn[:], q[:], pb[:], op=OP.add)
                u = work.tile([128, 16], BF16, tag=f"u{c}", name=f"u{c}_{t}")
                nc.scalar.activation(u[:], cn[:], AF.Sigmoid, scale=4.0)
                hh = work.tile([128, 16], BF16, tag=f"h{c}", name=f"h{c}_{t}")
                nc.vector.scalar_tensor_tensor(
                    hh[:], u[:], 0.5, so, op0=OP.subtract, op1=OP.mult)
                tp = tpool.tile([16, NB], BF16, tag=f"t{c}", name=f"tp{c}_{t}")
                nc.tensor.transpose(tp[:], hh[:], teye[:])
                nc.vector.tensor_scalar_add(
                    z[c][0:16, nsl * NB:(nsl + 1) * NB], tp[:], 0.0)
                c_cur[c] = cn

            # gather sigma(y) columns (slot s holds y_{t(s)-1}) to DRAM
            if t % 8 == 7:
                s0 = (t - 7) % RS   # always 0 or 8: contiguous run of 8
                for c in range(NCH):
                    if t == 7:  # slot 0 of chunk = y_{-1}: skip it
                        src = S[c][:, 1 * SW + 64:7 * SW + 65:SW]
                        dst = ot[c * NB:(c + 1) * NB, 0:7]
                    else:
                        src = S[c][:, (s0 * SW + 64):((s0 + 7) * SW + 65):SW]
                        dst = ot[c * NB:(c + 1) * NB, t - 8:t]
                    nc.sync.dma_start(dst, src)

        # trailing y_{t_steps-1} = sigma(2 wd h~_last + bd)
        fsl = t_steps % KSLOT
        for c in range(NCH):
            gt = gpool.tile([128, 1], F32, tag=f"g{c}", name=f"gt{c}")
            nc.tensor.matmul(gt[:], z[c][:, fsl * NB:(fsl + 1) * NB],
                             twg[:, 64:65], start=True, stop=True)
            st = work.tile([128, 1], BF16, tag=f"q{c}", name=f"st{c}")
            nc.scalar.activation(st[:], gt[:], AF.Sigmoid)
            nc.sync.dma_start(ot[c * NB:(c + 1) * NB, t_steps - 1:t_steps], st[:])

        for c in range(NCH):
            nc.sync.dma_start(h_out[:, c * NB:(c + 1) * NB],
                              z[c][0:16, fsl * NB:(fsl + 1) * NB])
            nc.sync.dma_start(c_out[c * NB:(c + 1) * NB, :], c_cur[c][:])


def _prep_host(W_ih, W_hh, b_ih, b_hh, W_d, b_d):
    # PyTorch gate order blocks of 16: [i, f, g, o]
    Wi, Wf, Wgt, Wo = W_ih[0:16], W_ih[16:32], W_ih[32:48], W_ih[48:64]
    Ui, Uf, Ugt, Uo = W_hh[0:16], W_hh[16:32], W_hh[32:48], W_hh[48:64]
    bb = b_ih + b_hh
    bi, bf, bgt, bo = bb[0:16], bb[16:32], bb[32:48], bb[48:64]

    wg = np.zeros((KD, GCOL), np.float32)

    def put(base, Wx, Ux, bx, scale):
        wg[0:16, base:base + 16] = (2.0 * scale) * Ux.T   # h~ = h/2
        wg[16:20, base:base + 16] = scale * Wx.T
        wg[20, base:base + 16] = scale * bx

    put(0, Wf, Uf, bf, 1.0)
    put(16, Wi, Ui, bi, 1.0)
    put(32, Wo, Uo, bo, 1.0)
    put(48, Wgt, Ugt, bgt, 2.0)   # sigma(2 glin)
    wg[0:16, 64] = 2.0 * W_d[0]   # y = 2 wd h~ + bd
    wg[20, 64] = float(b_d[0])
    return wg


def _get_compiled(t_steps):
    key = ("nc", t_steps)
    if key not in _CACHE:
        nc = bacc.Bacc("TRN2", target_bir_lowering=False, debug=False)
        _emit_core(nc, t_steps)
        nc.compile()
        _CACHE[key] = nc
    return _CACHE[key]


def kernel(x, W_ih, W_hh, b_ih, b_hh, W_d, b_d, _trace=False, _t_steps=T):
    import ml_dtypes
    from concourse.bass_utils import run_bass_kernel_spmd

    x = np.asarray(x, dtype=np.float32)
    ts = _t_steps
    wg = _prep_host(
        np.asarray(W_ih, np.float32), np.asarray(W_hh, np.float32),
        np.asarray(b_ih, np.float32), np.asarray(b_hh, np.float32),
        np.asarray(W_d, np.float32), np.asarray(b_d, np.float32))
    wg16 = wg.astype(ml_dtypes.bfloat16)
    eye16 = np.eye(128, dtype=ml_dtypes.bfloat16)

    # x [B, ts, I] -> [ts, I, B] bf16
    xtr16 = np.ascontiguousarray(
        x[:, 0:ts, :].transpose(1, 2, 0)).astype(ml_dtypes.bfloat16)

    CH = 512 if ts % 512 == 0 else ts
    nchunk = ts // CH
    nc = _get_compiled(CH)
    _ONES = np.ones((1, KSLOT * NB), ml_dtypes.bfloat16)
    h_st = [np.zeros((16, BCORE), ml_dtypes.bfloat16) for _ in range(NCORES)]
    c_st = [np.zeros((BCORE, 16), np.float32) for _ in range(NCORES)]
    out = np.empty((B, ts, 1), np.float32)
    total_ns = 0
    for ck in range(nchunk):
        in_maps = []
        for cix in range(NCORES):
            in_maps.append({
                "wg": wg16, "eye": eye16, "ones": _ONES,
                "h_in": h_st[cix], "c_in": c_st[cix],
                "xt": np.ascontiguousarray(
                    xtr16[ck * CH:(ck + 1) * CH, :,
                          cix * BCORE:(cix + 1) * BCORE]),
            })
        res = run_bass_kernel_spmd(nc, in_maps, core_ids=list(range(NCORES)),
                                   trace=_trace)
        for cix in range(NCORES):
            out[cix * BCORE:(cix + 1) * BCORE,
                ck * CH:(ck + 1) * CH, 0] = res.results[cix]["ot"]
            h_st[cix] = res.results[cix]["h_out"]
            c_st[cix] = res.results[cix]["c_out"]
        if res.exec_time_ns:
            total_ns += res.exec_time_ns
    kernel._last_exec_ns = total_ns or None
    return out


# revision 16
# speedup vs baseline: 1.1365x; 1.0023x over previous
"""Trainium2 Bass kernel: LSTM (B=2048, T=1024, I=4, H=16) + sigmoid dense head.

Sharding: pure data parallel, batch split over 8 cores (256 each = 2 chains x 128).

Batch-on-partitions orientation: the gate matmul is z_slot^T @ W with the
z-ring slice [21, 128] as the *stationary* lhsT and the weight matrix
[21, 65] as the moving rhs, so gates land [128 batch, 65 gate-cols] in PSUM.
Every elementwise op is then a full-128-lane column-sliced op (base partition
0 everywhere: no partition-base legality issues, bf16 2x packing applies) and
ONE sigmoid ACT covers all 4 gates + the dense-head pre-activation
y = 2*W_d h~ + b_d (rhs col 64, zero extra ops).

Per chain-step: MM -> ACT sigma_all -> DVE q=(sg-.5)*si -> DVE pb=sf*c ->
DVE cn=q+pb -> ACT u=sigma(4c~) -> DVE h~=(u-.5)*so -> PE transpose
[128,16]->[16,128] -> DVE copy PSUM->SBUF z-ring.

sigma(y) columns are DMA-gathered from the bf16 sigma-ring straight to DRAM
(batch-major ot [BCORE, T]) once per 8 steps.
State scalings: c~ = c/2, h~ = h/2 (absorbed into weights); tanh via
sigma(2x) identities so only the Sigmoid LUT is ever used.
"""
import sys
sys.path.insert(0, "/opt/trn_rl_repo")
import numpy as np
from contextlib import ExitStack

import concourse.bass as bass
import concourse.tile as tile
from concourse import bacc, mybir

F32 = mybir.dt.float32
BF16 = mybir.dt.bfloat16
AF = mybir.ActivationFunctionType
OP = mybir.AluOpType

B, T, I, H = 2048, 1024, 4, 16
NCORES = 8
BCORE = B // NCORES          # 256
NB = 128                     # batch per chain
NCH = 2                      # chains per core
KD = 21                      # z rows: 16 h~ + 4 x + 1 ones
GCOL = 65                    # rhs cols: f@0 i@16 o@32 g@48 y@64
SW = 66                      # sigma ring slot width (pad for 4B alignment)
KSLOT = 257                  # Z ring slots (2*STAGE+1)
STAGE = 128                  # x staging granularity (steps)
RS = 16                      # sigma ring slots (y gather groups of 8)

_CACHE = {}


def _emit_core(nc, t_steps):
    wg = nc.dram_tensor("wg", [KD, GCOL], BF16, kind="ExternalInput").ap()
    eye = nc.dram_tensor("eye", [128, 128], BF16, kind="ExternalInput").ap()
    xt = nc.dram_tensor("xt", [t_steps, I, BCORE], BF16, kind="ExternalInput").ap()
    ones = nc.dram_tensor("ones", [1, KSLOT * NB], BF16, kind="ExternalInput").ap()
    h_in = nc.dram_tensor("h_in", [16, BCORE], BF16, kind="ExternalInput").ap()
    c_in = nc.dram_tensor("c_in", [BCORE, 16], F32, kind="ExternalInput").ap()
    h_out = nc.dram_tensor("h_out", [16, BCORE], BF16, kind="ExternalOutput").ap()
    c_out = nc.dram_tensor("c_out", [BCORE, 16], F32, kind="ExternalOutput").ap()
    ot = nc.dram_tensor("ot", [BCORE, t_steps], BF16, kind="ExternalOutput").ap()

    with tile.TileContext(nc) as tc, ExitStack() as ctx:
        const = ctx.enter_context(tc.tile_pool(name="const", bufs=1))
        zpool = ctx.enter_context(tc.tile_pool(name="zp", bufs=1))
        spool = ctx.enter_context(tc.tile_pool(name="sp", bufs=1))
        work = ctx.enter_context(tc.tile_pool(name="wk", bufs=4))
        gpool = ctx.enter_context(tc.tile_pool(name="gp", bufs=2, space="PSUM"))
        tpool = ctx.enter_context(tc.tile_pool(name="tp", bufs=2, space="PSUM"))

        twg = const.tile([KD, GCOL], BF16)
        teye = const.tile([128, 128], BF16)
        nc.sync.dma_start(twg[:], wg[:])
        nc.sync.dma_start(teye[:], eye[:])

        # Z rings: rows 0:16 h~ (bf16), rows 16:20 x, row 20 ones
        z = [zpool.tile([KD, KSLOT * NB], BF16, name=f"z{c}") for c in range(NCH)]
        for c in range(NCH):
            nc.sync.dma_start(z[c][0:16, 0:NB], h_in[:, c * NB:(c + 1) * NB])
            nc.sync.dma_start(z[c][20:21, :], ones[:])

        # sigma rings: [128 batch, RS slots x 66 cols]; cols f i o g y pad
        S = [spool.tile([128, RS * SW], BF16, name=f"s{c}") for c in range(NCH)]

        c_cur = []
        for c in range(NCH):
            ci = work.tile([128, 16], F32, tag=f"c{c}", name=f"ci{c}")
            nc.sync.dma_start(ci[:], c_in[c * NB:(c + 1) * NB, :])
            c_cur.append(ci)

        def stage_x(c, t0, nsteps):
            s0 = t0 % KSLOT
            runs = []
            if s0 + nsteps <= KSLOT:
                runs.append((s0, t0, nsteps))
            else:
                n1 = KSLOT - s0
                runs.append((s0, t0, n1))
                runs.append((0, t0 + n1, nsteps - n1))
            for (sl, tt, ln) in runs:
                src = xt[tt:tt + ln, :, c * NB:(c + 1) * NB].rearrange("t i b -> i t b")
                dst = z[c][16:20, sl * NB:(sl + ln) * NB].rearrange(
                    "i (s b) -> i s b", s=ln)
                nc.sync.dma_start(dst, src)

        for c in range(NCH):
            stage_x(c, 0, min(STAGE, t_steps))

        for t in range(t_steps):
            if t % STAGE == 0 and t + STAGE < t_steps:
                for c in range(NCH):
                    stage_x(c, t + STAGE, min(STAGE, t_steps - t - STAGE))
            sl = t % KSLOT
            nsl = (t + 1) % KSLOT
            ss = t % RS
            # phase-sorted emission: engine queues are strict FIFO (except
            # PE), so order per engine must match data-readiness order
            gt_ = []
            sf, si, so, sg, qt, pbt, cnt, ut, hht = ({} for _ in range(9))
            for c in range(NCH):
                g = gpool.tile([128, GCOL], F32, tag=f"g{c}", name=f"g{c}_{t}")
                nc.tensor.matmul(g[:], z[c][:, sl * NB:(sl + 1) * NB], twg[:],
                                 start=True, stop=True)
                gt_.append(g)
            for c in range(NCH):
                sv = S[c][:, ss * SW:ss * SW + GCOL]
                nc.scalar.activation(sv[:], gt_[c][:], AF.Sigmoid)
                sf[c] = S[c][:, ss * SW + 0:ss * SW + 16]
                si[c] = S[c][:, ss * SW + 16:ss * SW + 32]
                so[c] = S[c][:, ss * SW + 32:ss * SW + 48]
                sg[c] = S[c][:, ss * SW + 48:ss * SW + 64]
            for c in range(NCH):
                qt[c] = work.tile([128, 16], BF16, tag=f"q{c}", name=f"q{c}_{t}")
                nc.vector.scalar_tensor_tensor(
                    qt[c][:], sg[c], 0.5, si[c], op0=OP.subtract, op1=OP.mult)
            for c in range(NCH):
                pbt[c] = work.tile([128, 16], F32, tag=f"p{c}", name=f"p{c}_{t}")
                nc.gpsimd.tensor_tensor(
                    pbt[c][:], sf[c], c_cur[c][:], op=OP.mult)
            for c in range(NCH):
                cnt[c] = work.tile([128, 16], F32, tag=f"c{c}", name=f"cn{c}_{t}")
                nc.gpsimd.tensor_tensor(cnt[c][:], qt[c][:], pbt[c][:], op=OP.add)
            for c in range(NCH):
                ut[c] = work.tile([128, 16], BF16, tag=f"u{c}", name=f"u{c}_{t}")
                nc.scalar.activation(ut[c][:], cnt[c][:], AF.Sigmoid, scale=4.0)
            for c in range(NCH):
                hht[c] = work.tile([128, 16], BF16, tag=f"h{c}", name=f"h{c}_{t}")
                nc.vector.scalar_tensor_tensor(
                    hht[c][:], ut[c][:], 0.5, so[c], op0=OP.subtract, op1=OP.mult)
            tps = []
            for c in range(NCH):
                tp = tpool.tile([16, NB], BF16, tag=f"t{c}", name=f"tp{c}_{t}")
                nc.tensor.transpose(tp[:], hht[c][:], teye[:])
                tps.append(tp)
            for c in range(NCH):
                nc.vector.tensor_scalar_add(
                    z[c][0:16, nsl * NB:(nsl + 1) * NB], tps[c][:], 0.0)
                c_cur[c] = cnt[c]

            # gather sigma(y) columns (slot s holds y_{t(s)-1}) to DRAM
            if t % 8 == 7:
                s0 = (t - 7) % RS   # always 0 or 8: contiguous run of 8
                for c in range(NCH):
                    if t == 7:  # slot 0 of chunk = y_{-1}: skip it
                        src = S[c][:, 1 * SW + 64:7 * SW + 65:SW]
                        dst = ot[c * NB:(c + 1) * NB, 0:7]
                    else:
                        src = S[c][:, (s0 * SW + 64):((s0 + 7) * SW + 65):SW]
                        dst = ot[c * NB:(c + 1) * NB, t - 8:t]
                    nc.sync.dma_start(dst, src)

        # trailing y_{t_steps-1} = sigma(2 wd h~_last + bd)
        fsl = t_steps % KSLOT
        for c in range(NCH):
            gt = gpool.tile([128, 1], F32, tag=f"g{c}", name=f"gt{c}")
            nc.tensor.matmul(gt[:], z[c][:, fsl * NB:(fsl + 1) * NB],
                             twg[:, 64:65], start=True, stop=True)
            st = work.tile([128, 1], BF16, tag=f"q{c}", name=f"st{c}")
            nc.scalar.activation(st[:], gt[:], AF.Sigmoid)
            nc.sync.dma_start(ot[c * NB:(c + 1) * NB, t_steps - 1:t_steps], st[:])

        for c in range(NCH):
            nc.sync.dma_start(h_out[:, c * NB:(c + 1) * NB],
                              z[c][0:16, fsl * NB:(fsl + 1) * NB])
            nc.sync.dma_start(c_out[c * NB:(c + 1) * NB, :], c_cur[c][:])


def _prep_host(W_ih, W_hh, b_ih, b_hh, W_d, b_d):
    # PyTorch gate order blocks of 16: [i, f, g, o]
    Wi, Wf, Wgt, Wo = W_ih[0:16], W_ih[16:32], W_ih[32:48], W_ih[48:64]
    Ui, Uf, Ugt, Uo = W_hh[0:16], W_hh[16:32], W_hh[32:48], W_hh[48:64]
    bb = b_ih + b_hh
    bi, bf, bgt, bo = bb[0:16], bb[16:32], bb[32:48], bb[48:64]

    wg = np.zeros((KD, GCOL), np.float32)

    def put(base, Wx, Ux, bx, scale):
        wg[0:16, base:base + 16] = (2.0 * scale) * Ux.T   # h~ = h/2
        wg[16:20, base:base + 16] = scale * Wx.T
        wg[20, base:base + 16] = scale * bx

    put(0, Wf, Uf, bf, 1.0)
    put(16, Wi, Ui, bi, 1.0)
    put(32, Wo, Uo, bo, 1.0)
    put(48, Wgt, Ugt, bgt, 2.0)   # sigma(2 glin)
    wg[0:16, 64] = 2.0 * W_d[0]   # y = 2 wd h~ + bd
    wg[20, 64] = float(b_d[0])
    return wg


def _get_compiled(t_steps):
    key = ("nc", t_steps)
    if key not in _CACHE:
        nc = bacc.Bacc("TRN2", target_bir_lowering=False, debug=False)
        _emit_core(nc, t_steps)
        nc.compile()
        _CACHE[key] = nc
    return _CACHE[key]


def kernel(x, W_ih, W_hh, b_ih, b_hh, W_d, b_d, _trace=False, _t_steps=T):
    import ml_dtypes
    from concourse.bass_utils import run_bass_kernel_spmd

    x = np.asarray(x, dtype=np.float32)
    ts = _t_steps
    wg = _prep_host(
        np.asarray(W_ih, np.float32), np.asarray(W_hh, np.float32),
        np.asarray(b_ih, np.float32), np.asarray(b_hh, np.float32),
        np.asarray(W_d, np.float32), np.asarray(b_d, np.float32))
    wg16 = wg.astype(ml_dtypes.bfloat16)
    eye16 = np.eye(128, dtype=ml_dtypes.bfloat16)

    # x [B, ts, I] -> [ts, I, B] bf16
    xtr16 = np.ascontiguousarray(
        x[:, 0:ts, :].transpose(1, 2, 0)).astype(ml_dtypes.bfloat16)

    CH = 512 if ts % 512 == 0 else ts
    nchunk = ts // CH
    nc = _get_compiled(CH)
    _ONES = np.ones((1, KSLOT * NB), ml_dtypes.bfloat16)
    h_st = [np.zeros((16, BCORE), ml_dtypes.bfloat16) for _ in range(NCORES)]
    c_st = [np.zeros((BCORE, 16), np.float32) for _ in range(NCORES)]
    out = np.empty((B, ts, 1), np.float32)
    total_ns = 0
    for ck in range(nchunk):
        in_maps = []
        for cix in range(NCORES):
            in_maps.append({
                "wg": wg16, "eye": eye16, "ones": _ONES,
                "h_in": h_st[cix], "c_in": c_st[cix],
                "xt": np.ascontiguousarray(
                    xtr16[ck * CH:(ck + 1) * CH, :,
                          cix * BCORE:(cix + 1) * BCORE]),
            })
        res = run_bass_kernel_spmd(nc, in_maps, core_ids=list(range(NCORES)),
                                   trace=_trace)
        for cix in range(NCORES):
            out[cix * BCORE:(cix + 1) * BCORE,
                ck * CH:(ck + 1) * CH, 0] = res.results[cix]["ot"]
            h_st[cix] = res.results[cix]["h_out"]
            c_st[cix] = res.results[cix]["c_out"]
        if res.exec_time_ns:
            total_ns += res.exec_time_ns
    kernel._last_exec_ns = total_ns or None
    return out
